# revision 13
# baseline (speedup 1.0000x reference)
"""Additive (Bahdanau) attention on 8 TRN2 NeuronCores (raw Bass).

Reference math (B=4, Tq=256, Tk=512, Dq=Dv=512, U=256):
    q = query @ W1                      [B,Tq,U]
    k = value @ W2                      [B,Tk,U]
    scores[b,t,s] = sum_u scale[u] * tanh(q[b,t,u] + k[b,s,u])
    attn = softmax(scores, axis=-1)     [B,Tq,Tk]
    context = attn @ value              [B,Tq,Dv]
    returns (context, attn)

Sharding: (b, tq-half) -> 8 cores, 128 query rows each; Tk local.

Algorithm (per core): tanh(x) ~= alpha*x + sum_{m odd<=9} c_m sin(m w x),
each sine separable -> 20 PE matmuls over u + exact linear term computed
as (alpha W2 scale) . v^T (4 matmuls, no k copy needed).  Harmonic factor
tensors built on DVE via Chebyshev step-2 recurrences over CONCATENATED
[sin|cos] buffers (halves op count; K ops are [128,2048], Q ops [128,512])
with base sin/cos on ACT (args < pi).  m=3 via (C4+-1) * base (D-trick).
Q side is pre-scaled by scale_u (S4 const) so taps are pure-imm scalings;
taps m=1,3 on DVE, m=5,7,9 on ACT (engine balance).

Schedule: PE warm-up matmul burst during the initial DMA wait flips the
HAM clock gate (1.2 -> 2.4 GHz) before real matmuls; keep-warm dummies
between score rounds.  Input DMAs are spread over 4 issue queues with
criticals (w2, vT) first.  Outputs in fp16 (halved DMA).
"""

from contextlib import ExitStack

import numpy as np

import concourse.bass as bass
import concourse.mybir as mybir
from concourse.bass_utils import run_bass_kernel_spmd

F32 = mybir.dt.float32
BF16 = mybir.dt.bfloat16
FP16 = mybir.dt.float16
I32 = mybir.dt.int32
AF = mybir.ActivationFunctionType
ALU = mybir.AluOpType

N_CORES = 8
B, TQ, TK, DQ, DV, U = 4, 256, 512, 512, 512, 256
T_ROWS = 128
UC = U // 128          # 2
DC = DQ // 128         # 4
SC = TK // 128         # 4

# tanh(x) ~= ALPHA*x + sum c_m sin(m pi x / L), m odd, |x|<=9.4
L_PER = 10.1
OMEGA = float(np.pi / L_PER)
ALPHA = 0.01535833903650663
MS = [1, 3, 5, 7, 9]
COEFS = [1.159928933795801, 0.30947520157694797, 0.11021886920747345,
         0.036057634532860455, 0.022497063758245502]
HALF_PI = float(np.pi / 2)

KW = UC * TK           # 1024 (K-side half-width)
QW = UC * 128          # 256  (Q-side half-width)
N_WARM = 18


def build_bass(debug: bool = False, race: bool = True) -> bass.Bass:
    nc = bass.Bass(detect_race_conditions=race)
    w2a_ext = nc.declare_dram_parameter("w2a", [128, 2 * U], FP16, isOutput=False)
    w2b_ext = nc.declare_dram_parameter("w2bx", [128, 2 * U], FP16, isOutput=False)
    vt_ext = [
        nc.declare_dram_parameter(f"vt{dc}", [128, TK], FP16, isOutput=False)
        for dc in range(DC)
    ]
    qt_ext = nc.declare_dram_parameter("qth", [128, DC * 128], FP16, isOutput=False)
    w1_ext = nc.declare_dram_parameter("w1h", [128, DC * U], FP16, isOutput=False)
    vb_ext = nc.declare_dram_parameter("vbb", [128, SC * DV], BF16, isOutput=False)
    U8 = mybir.dt.uint8
    CONST_W = 4 + 2 * 2 * QW + 2 * 128 + 2 * DC * 128   # hpi | S4 | idb | w2sb
    cst_ext = nc.declare_dram_parameter("constb", [128, CONST_W], U8, isOutput=False)
    ctx_ext = nc.declare_dram_parameter("context", [T_ROWS, DV], FP16, isOutput=True)
    attn_ext = nc.declare_dram_parameter("attn", [T_ROWS, TK], FP16, isOutput=True)
    dbg_ext = {}
    if debug:
        for nm, w, dt in [
            ("dbg_k1", 2 * KW, FP16), ("dbg_k3", 2 * KW, FP16),
            ("dbg_k9", 2 * KW, FP16), ("dbg_q1p", 2 * QW, FP16),
            ("dbg_q9", 2 * QW, FP16), ("dbg_a5", 2 * QW, FP16),
            ("dbg_e", TK, BF16), ("dbg_sums", 1, F32),
            ("dbg_etb", TK, BF16),
        ]:
            dbg_ext[nm] = nc.declare_dram_parameter(nm, [128, w], dt, isOutput=True)

    es = ExitStack()
    with es:
        _n = [0]

        def sb(shape, dt):
            _n[0] += 1
            return es.enter_context(nc.sbuf_tensor(f"sb{_n[0]}", shape, dt))

        # ---- SBUF ----
        w2b = sb([128, DC * U], FP16)
        vtb = sb([128, DC * TK], FP16)
        qtb = sb([128, DC * 128], FP16)
        w1b = sb([128, DC * U], FP16)
        vbf = sb([128, SC * DV], BF16)
        cstb = sb([128, CONST_W], U8)
        hpi = cstb[:, 0:4].bitcast(F32)                          # [128, 1]
        s4 = cstb[:, 4 : 4 + 2 * 2 * QW].bitcast(FP16)           # [128, 512]
        idb = cstb[:, 1028:1284].bitcast(BF16)                   # [128, 128]
        w2sb = cstb[:, 1284:2308].bitcast(FP16)                  # [128, 512]
        wz = sb([128, 128], FP16)          # warm-up zeros
        # K-side concat factor tensors [u_p, sin(uc0|uc1) | cos(uc0|uc1)]
        K = {m: sb([128, 2 * KW], FP16) for m in MS}
        tmpK = sb([128, KW], FP16)
        C4K = sb([128, 2 * KW], FP16)      # [C4 | C4]
        DKc = sb([128, 2 * KW], FP16)      # [C4+1 | C4-1]
        tK = sb([128, 2 * KW], FP16)       # round scratch
        # Q side [u_p, sin | cos], prescaled by scale_u after m=1
        Q = {m: sb([128, 2 * QW], FP16) for m in MS}   # Q[1] raw; Q1P scaled
        Q1P = sb([128, 2 * QW], FP16)
        tmpQ = sb([128, QW], FP16)
        C4Q = sb([128, 2 * QW], FP16)
        DQc = sb([128, 2 * QW], FP16)
        tQ = sb([128, 2 * QW], FP16)
        A = {m: sb([128, 2 * QW], FP16) for m in MS}   # taps c_m * Q[m]
        E_bf = sb([128, TK], BF16)
        sums = sb([128, 1], F32)
        sums0 = sb([128, 1], F32)
        sums1 = sb([128, 1], F32)
        scr1 = sb([128, 1], F32)
        r_sb = sb([128, 1], F32)
        ETb = sb([128, TK], BF16)
        attn_h = sb([128, TK], FP16)
        ctx_h = sb([128, DV], FP16)

        # ---- PSUM (8 banks x 2KB) ----
        ringA = es.enter_context(nc.psum_tensor("ringA", [128, 2048], F32))
        ringB = es.enter_context(nc.psum_tensor("ringB", [128, 1536], F32))
        kps = ringA[:, 0:1024]                 # k proj [u, (uc,s)]
        scores_ps = ringA[:, 1024:1536]
        qps = ringA[:, 1536:1792]              # q proj [u, (uc,t)]
        warm_ps = ringB[:, 1024:1152]
        etps_bf = ringB[:, 0:256].bitcast(BF16)   # [128, 512] bf16 view
        etps_i32 = ringB[:, 0:256].bitcast(I32)   # [128, 256] i32 view
        ctxps = ringB[:, 512:1024]
        ETb_i32 = ETb.bitcast(I32)                # [128, 256]

        sem = lambda name: es.enter_context(nc.semaphore(name))
        s_w2a = sem("s_w2a")
        s_w2b = sem("s_w2b")
        s_vt = [sem(f"s_vt{dc}") for dc in range(DC)]
        s_qt = sem("s_qt")
        s_w1 = sem("s_w1")
        s_vbf = sem("s_vbf")
        s_cst = sem("s_cst")
        s_warm = sem("s_warm")
        s_kp = sem("s_kp")       # 2
        s_qp = sem("s_qp")       # 2
        s_act = sem("s_act")     # ACT bases: XsK1, XcK1, tq_s, tq_c
        s_tapD = sem("s_tapD")   # DVE taps A1, A3
        s_qr = sem("s_qr")       # DVE round completions (Q5, Q7, Q9)
        s_tapA = sem("s_tapA")   # ACT taps A5, A7, A9
        s_scores = sem("s_scores")
        s_exp = sem("s_exp")
        s_transp = sem("s_transp")  # 4
        s_etb = sem("s_etb")        # 2
        s_recip = sem("s_recip")
        s_ctxmm = sem("s_ctxmm")
        s_att = sem("s_att")
        s_ctxo = sem("s_ctxo")
        s_dout = sem("s_dout")

        c_of = {m: float(c) for m, c in zip(MS, COEFS)}

        with nc.Block() as block:

            @block.sync
            def _(sync):
                sync.dma_start(out=vtb[:, 0:512], in_=vt_ext[0][:, :]).then_inc(
                    s_vt[0], 16
                )
                sync.dma_start(
                    out=vtb[:, 1024:1536], in_=vt_ext[2][:, :]
                ).then_inc(s_vt[2], 16)
                sync.wait_ge(s_att, 1)
                sync.dma_start(out=attn_ext[:, :], in_=attn_h[:, :]).then_inc(
                    s_dout, 16
                )

                if debug:
                    sync.wait_ge(s_ctxo, 1)
                    dbg_srcs = {
                        "dbg_k1": K[1], "dbg_k3": K[3], "dbg_k9": K[9],
                        "dbg_q1p": Q1P, "dbg_q9": Q[9], "dbg_a5": A[5],
                        "dbg_e": E_bf, "dbg_sums": sums, "dbg_etb": ETb,
                    }
                    for nm, src in dbg_srcs.items():
                        sync.dma_start(
                            out=dbg_ext[nm][:, :], in_=src[:, :]
                        ).then_inc(s_dout, 16)
                    sync.wait_ge(s_dout, 32 + 16 * len(dbg_srcs))
                else:
                    sync.wait_ge(s_dout, 32)

            @block.scalar
            def _(scalar):
                scalar.dma_start(out=vtb[:, 512:1024], in_=vt_ext[1][:, :]).then_inc(
                    s_vt[1], 16
                )
                scalar.dma_start(
                    out=vtb[:, 1536:2048], in_=vt_ext[3][:, :]
                ).then_inc(s_vt[3], 16)
                scalar.dma_start(out=cstb[:, :], in_=cst_ext[:, :]).then_inc(
                    s_cst, 16
                )
                # preload the Sin table during the DMA wait
                scalar.wait_ge(s_warm, 1)
                scalar.activation(out=scr1[:, 0:1], in_=wz[:, 0:1], func=AF.Sin)
                # K bases into concat halves (args < pi; pi/2-bias cos valid
                # since |w*x| <= pi/2 with margin)
                scalar.wait_ge(s_kp, 2)
                scalar.activation(
                    out=K[1][:, 0:KW], in_=kps, func=AF.Sin, scale=OMEGA
                ).then_inc(s_act, 1)  # 1
                scalar.wait_ge(s_cst, 16)
                scalar.activation(
                    out=K[1][:, KW : 2 * KW], in_=kps, func=AF.Sin, scale=OMEGA,
                    bias=hpi[:, 0:1],
                ).then_inc(s_act, 1)  # 2
                scalar.wait_ge(s_qp, 2)
                scalar.activation(
                    out=Q[1][:, 0:QW], in_=qps, func=AF.Sin, scale=OMEGA
                ).then_inc(s_act, 1)  # 3
                scalar.activation(
                    out=Q[1][:, QW : 2 * QW], in_=qps, func=AF.Sin, scale=OMEGA,
                    bias=hpi[:, 0:1],
                ).then_inc(s_act, 1)  # 4
                # preload the Exp table now (off the critical tail)
                scalar.activation(out=scr1[:, 0:1], in_=wz[:, 0:1], func=AF.Exp)
                # ACT taps for m=5,7,9 as DVE rounds complete
                for i, m in enumerate(MS[2:]):
                    scalar.wait_ge(s_qr, i + 1)
                    scalar.activation(
                        out=A[m][:, :], in_=Q[m][:, :], func=AF.Copy,
                        scale=c_of[m],
                    ).then_inc(s_tapA, 1)
                # softmax exp + row sums
                scalar.wait_ge(s_scores, 1)
                scalar.activation(
                    out=E_bf[:, :], in_=scores_ps, func=AF.Exp,
                    accum_out=sums[:, 0:1],
                ).then_inc(s_exp, 1)
                scalar.activation(
                    out=E_bf[:, 0:1], in_=E_bf[:, 0:1], func=AF.Copy,
                ).then_inc(s_exp, 1)
                # attn normalize (fp16 out)
                scalar.wait_ge(s_recip, 1)
                scalar.activation(
                    out=attn_h[:, :], in_=E_bf[:, :], func=AF.Copy,
                    scale=r_sb[:, 0:1],
                ).then_inc(s_att, 1)
                scalar.wait_ge(s_ctxo, 1)
                scalar.dma_start(out=ctx_ext[:, :], in_=ctx_h[:, :]).then_inc(
                    s_dout, 16
                )

            @block.gpsimd
            def _(gpsimd):
                gpsimd.dma_start(out=w2b[:, 0 : 2 * U], in_=w2a_ext[:, :]).then_inc(
                    s_w2a, 16
                )
                gpsimd.dma_start(
                    out=w2b[:, 2 * U : 4 * U], in_=w2b_ext[:, :]
                ).then_inc(s_w2b, 16)
                gpsimd.wait_ge(s_w2b, 16)
                gpsimd.dma_start(out=qtb[:, :], in_=qt_ext[:, :]).then_inc(s_qt, 16)
                gpsimd.dma_start(out=w1b[:, :], in_=w1_ext[:, :]).then_inc(s_w1, 16)
                gpsimd.wait_ge(s_qp, 2)
                gpsimd.dma_start(out=vbf[:, :], in_=vb_ext[:, :]).then_inc(
                    s_vbf, 16
                )

            @block.vector
            def _(vector):
                vector.memset(wz[:, :], 0.0)
                vector.drain()
                vector.sem_inc(s_warm, 1)
                # K-side prep
                vector.wait_ge(s_act, 1)
                vector.tensor_tensor(
                    out=tmpK[:, :], in0=K[1][:, 0:KW], in1=K[1][:, 0:KW],
                    op=ALU.mult,
                )
                vector.tensor_scalar(
                    out=C4K[:, 0:KW], in0=tmpK[:, :], scalar1=-4.0, scalar2=2.0,
                    op0=ALU.mult, op1=ALU.add,
                )
                vector.tensor_scalar(
                    out=C4K[:, KW : 2 * KW], in0=tmpK[:, :], scalar1=-4.0,
                    scalar2=2.0, op0=ALU.mult, op1=ALU.add,
                )
                vector.tensor_scalar_add(
                    out=DKc[:, 0:KW], in0=C4K[:, 0:KW], scalar1=1.0
                )
                vector.tensor_scalar_add(
                    out=DKc[:, KW : 2 * KW], in0=C4K[:, 0:KW], scalar1=-1.0
                )
                # m=3 K: [s3|c3] = [C4+1|C4-1] * [s1|c1]
                vector.wait_ge(s_act, 2)
                vector.tensor_tensor(
                    out=K[3][:, :], in0=DKc[:, :], in1=K[1][:, :], op=ALU.mult
                )
                # Q-side prep
                vector.wait_ge(s_act, 3)
                vector.tensor_tensor(
                    out=tmpQ[:, :], in0=Q[1][:, 0:QW], in1=Q[1][:, 0:QW],
                    op=ALU.mult,
                )
                vector.tensor_scalar(
                    out=C4Q[:, 0:QW], in0=tmpQ[:, :], scalar1=-4.0, scalar2=2.0,
                    op0=ALU.mult, op1=ALU.add,
                )
                vector.tensor_scalar(
                    out=C4Q[:, QW : 2 * QW], in0=tmpQ[:, :], scalar1=-4.0,
                    scalar2=2.0, op0=ALU.mult, op1=ALU.add,
                )
                vector.tensor_scalar_add(
                    out=DQc[:, 0:QW], in0=C4Q[:, 0:QW], scalar1=1.0
                )
                vector.tensor_scalar_add(
                    out=DQc[:, QW : 2 * QW], in0=C4Q[:, 0:QW], scalar1=-1.0
                )
                # prescale by scale_u (S4 const), tap m=1, m=3 chain + tap
                vector.wait_ge(s_act, 4)
                vector.wait_ge(s_cst, 16)
                vector.tensor_tensor(
                    out=Q1P[:, :], in0=Q[1][:, :], in1=s4[:, :], op=ALU.mult
                )
                vector.tensor_scalar_mul(
                    out=A[1][:, :], in0=Q1P[:, :], scalar1=c_of[1]
                ).then_inc(s_tapD, 1)
                vector.tensor_tensor(
                    out=Q[3][:, :], in0=DQc[:, :], in1=Q1P[:, :], op=ALU.mult
                )
                vector.tensor_scalar_mul(
                    out=A[3][:, :], in0=Q[3][:, :], scalar1=c_of[3]
                ).then_inc(s_tapD, 1)
                # rounds m = 5, 7, 9 on concat buffers
                for j, m in enumerate(MS[2:]):
                    p2, p4 = m - 2, m - 4
                    q_p4 = Q1P if p4 == 1 else Q[p4]
                    vector.tensor_tensor(
                        out=tK[:, :], in0=C4K[:, :], in1=K[p2][:, :], op=ALU.mult
                    )
                    vector.tensor_tensor(
                        out=tQ[:, :], in0=C4Q[:, :], in1=Q[p2][:, :], op=ALU.mult
                    )
                    vector.tensor_tensor(
                        out=K[m][:, :], in0=tK[:, :], in1=K[p4][:, :],
                        op=ALU.subtract,
                    )
                    vector.tensor_tensor(
                        out=Q[m][:, :], in0=tQ[:, :], in1=q_p4[:, :],
                        op=ALU.subtract,
                    ).then_inc(s_qr, 1)
                # 1/sums (ahead of evacs: unblocks the attn path)
                vector.wait_ge(s_exp, 2)
                vector.reciprocal(out=r_sb[:, :], in_=sums[:, :])
                vector.drain()
                vector.sem_inc(s_recip, 1)
                # ET evac (bf16 as int32: half the elements)
                vector.wait_ge(s_transp, 2)
                vector.tensor_copy(out=ETb_i32[:, 0:128], in_=etps_i32[:, 0:128])
                vector.drain()
                vector.sem_inc(s_etb, 1)
                vector.wait_ge(s_transp, 4)
                vector.tensor_copy(out=ETb_i32[:, 128:256], in_=etps_i32[:, 128:256])
                vector.drain()
                vector.sem_inc(s_etb, 1)
                # ctx normalize (fp16 out)
                vector.wait_ge(s_ctxmm, 1)
                vector.tensor_scalar_mul(
                    out=ctx_h[:, :], in0=ctxps, scalar1=r_sb[:, 0:1]
                ).then_inc(s_ctxo, 1)

            @block.tensor
            def _(tensor):
                # HAM warm-up burst on zeros during the input DMA wait
                tensor.wait_ge(s_warm, 1)
                for _ in range(N_WARM):
                    tensor.matmul(
                        out=warm_ps, lhsT=wz[:, :], rhs=wz[:, :],
                        start=True, stop=True,
                    )
                # k projection (interleaved uc groups, per-dc-pair waits)
                for dc in range(DC):
                    if dc == 0:
                        tensor.wait_ge(s_w2a, 16)
                    elif dc == 2:
                        tensor.wait_ge(s_w2b, 16)
                    tensor.wait_ge(s_vt[dc], 16)
                    for uc in range(UC):
                        ins = tensor.matmul(
                            out=kps[:, uc * TK : (uc + 1) * TK],
                            lhsT=w2b[:, dc * U + uc * 128 : dc * U + uc * 128 + 128],
                            rhs=vtb[:, dc * TK : (dc + 1) * TK],
                            start=(dc == 0),
                            stop=(dc == DC - 1),
                        )
                        if dc == DC - 1:
                            ins.then_inc(s_kp, 1)
                # q projection (uc groups sequential: one open group per bank)
                tensor.wait_ge(s_qt, 16)
                tensor.wait_ge(s_w1, 16)
                for uc in range(UC):
                    for dc in range(DC):
                        ins = tensor.matmul(
                            out=qps[:, uc * 128 : (uc + 1) * 128],
                            lhsT=w1b[:, dc * U + uc * 128 : dc * U + uc * 128 + 128],
                            rhs=qtb[:, dc * 128 : (dc + 1) * 128],
                            start=(dc == 0),
                            stop=(dc == DC - 1),
                        )
                    ins.then_inc(s_qp, 1)
                # scores: exact linear term first (w2s . vT, const along t)
                tensor.wait_ge(s_cst, 16)
                for dc in range(DC):
                    tensor.matmul(
                        out=scores_ps,
                        lhsT=w2sb[:, dc * 128 : (dc + 1) * 128],
                        rhs=vtb[:, dc * TK : (dc + 1) * TK],
                        start=(dc == 0),
                        stop=False,
                    )
                # harmonics: lhsT = A[m] tiles, rhs = K[m] slices
                for i, m in enumerate(MS):
                    # keep-warm dummies before the tap wait
                    for _ in range(2):
                        tensor.matmul(
                            out=warm_ps, lhsT=wz[:, :], rhs=wz[:, :],
                            start=True, stop=True,
                        )
                    if m == 1:
                        tensor.wait_ge(s_act, 2)
                        tensor.wait_ge(s_tapD, 1)
                    elif m == 3:
                        tensor.wait_ge(s_tapD, 2)
                    else:
                        tensor.wait_ge(s_tapA, i - 1)
                    # A layout [As_uc0|As_uc1|Ac_uc0|Ac_uc1]
                    # K layout [Ks_uc0|Ks_uc1|Kc_uc0|Kc_uc1] (512 each)
                    for kind in range(2):   # 0: As x Kc, 1: Ac x Ks
                        for uc in range(UC):
                            lhs = A[m][:, (kind * 2 + uc) * 128 :
                                       (kind * 2 + uc) * 128 + 128]
                            rhs_off = (1 - kind) * KW + uc * TK
                            last = (m == MS[-1]) and (kind == 1) and (uc == UC - 1)
                            ins = tensor.matmul(
                                out=scores_ps,
                                lhsT=lhs,
                                rhs=K[m][:, rhs_off : rhs_off + TK],
                                start=False,
                                stop=last,
                            )
                            if last:
                                ins.then_inc(s_scores, 1)
                # transposes of E for ctx (per exp chunk)
                tensor.matmul(
                    out=warm_ps, lhsT=wz[:, :], rhs=wz[:, :], start=True, stop=True
                )
                tensor.matmul(
                    out=warm_ps, lhsT=wz[:, :], rhs=wz[:, :], start=True, stop=True
                )
                for sc in range(SC):
                    if sc == 0:
                        tensor.wait_ge(s_exp, 1)
                    elif sc == 2:
                        tensor.wait_ge(s_exp, 2)
                    tensor.transpose(
                        out=etps_bf[:, sc * 128 : (sc + 1) * 128],
                        in_=E_bf[:, sc * 128 : (sc + 1) * 128],
                        identity=idb[:, :],
                    ).then_inc(s_transp, 1)
                # context (chunk-pipelined on ET evacs)
                tensor.wait_ge(s_vbf, 16)
                for sc in range(SC):
                    if sc == 0:
                        tensor.wait_ge(s_etb, 1)
                    elif sc == 2:
                        tensor.wait_ge(s_etb, 2)
                    ins = tensor.matmul(
                        out=ctxps,
                        lhsT=ETb[:, sc * 128 : (sc + 1) * 128],
                        rhs=vbf[:, sc * DV : (sc + 1) * DV],
                        start=(sc == 0),
                        stop=(sc == SC - 1),
                    )
                    if sc == SC - 1:
                        ins.then_inc(s_ctxmm, 1)

    return nc


_NC = None


def _get_nc() -> bass.Bass:
    global _NC
    if _NC is None:
        _NC = build_bass()
    return _NC


def make_in_maps(query, value, W1, W2, scale):
    import ml_dtypes

    bf = ml_dtypes.bfloat16
    fh = np.float16
    query = np.asarray(query, dtype=np.float32)
    value = np.asarray(value, dtype=np.float32)
    W1 = np.asarray(W1, np.float32)
    W2 = np.asarray(W2, np.float32)
    scale = np.asarray(scale, np.float32)
    # pack [D, X] operands into SBUF layout [128, (chunk, x)]
    pk = lambda a: np.ascontiguousarray(
        a.reshape(4, 128, a.shape[1]).transpose(1, 0, 2).reshape(128, -1)
    )
    w1h = pk(W1.astype(fh))
    w2h = pk(W2.astype(fh))
    # S4: [scl_uc0 | scl_uc1 | scl_uc0 | scl_uc1] each column-bcast [128,128]
    scl2 = scale.reshape(UC, 128).astype(fh)          # [uc, up]
    s4 = np.concatenate(
        [np.broadcast_to(scl2[uc][:, None], (128, 128)) for uc in (0, 1)] * 2,
        axis=1,
    )
    # w2sb: lhsT rows = alpha * (W2 @ scale) chunked by dc, bcast along t
    w2s = (ALPHA * (W2 @ scale)).astype(fh).reshape(DC, 128)
    w2sb = np.concatenate(
        [np.broadcast_to(w2s[dc][:, None], (128, 128)) for dc in range(DC)],
        axis=1,
    )
    constb = np.ascontiguousarray(
        np.concatenate(
            [
                np.full((128, 1), np.pi / 2, np.float32).view(np.uint8),
                s4.astype(fh).view(np.uint8),
                np.eye(128).astype(bf).view(np.uint8),
                w2sb.astype(fh).view(np.uint8),
            ],
            axis=1,
        )
    )
    in_maps = []
    for c in range(N_CORES):
        b, th = c // 2, c % 2
        qloc = query[b, th * T_ROWS : (th + 1) * T_ROWS, :]
        vloc = value[b]
        vth = pk(vloc.T.astype(fh))
        im = {
            "w2a": np.ascontiguousarray(w2h[:, 0:512]),
            "w2bx": np.ascontiguousarray(w2h[:, 512:1024]),
            "qth": pk(qloc.T.astype(fh)),
            "w1h": w1h,
            "vbb": pk(vloc.astype(bf)),
            "constb": constb,
        }
        for dc in range(DC):
            im[f"vt{dc}"] = np.ascontiguousarray(vth[:, dc * 512 : (dc + 1) * 512])
        in_maps.append(im)
    return in_maps


def assemble(results):
    context = np.empty((B, TQ, DV), dtype=np.float32)
    attn = np.empty((B, TQ, TK), dtype=np.float32)
    for c in range(N_CORES):
        b, th = c // 2, c % 2
        context[b, th * T_ROWS : (th + 1) * T_ROWS, :] = results[c][
            "context"
        ].astype(np.float32)
        attn[b, th * T_ROWS : (th + 1) * T_ROWS, :] = results[c]["attn"].astype(
            np.float32
        )
    return context, attn


def kernel(query, value, W1, W2, scale):
    nc = _get_nc()
    in_maps = make_in_maps(query, value, W1, W2, scale)
    res = run_bass_kernel_spmd(nc, in_maps, core_ids=list(range(N_CORES)))
    return assemble(res.results)


# revision 14
# speedup vs baseline: 1.0271x; 1.0271x over previous
"""Additive (Bahdanau) attention on 8 TRN2 NeuronCores (raw Bass).

Reference math (B=4, Tq=256, Tk=512, Dq=Dv=512, U=256):
    q = query @ W1                      [B,Tq,U]
    k = value @ W2                      [B,Tk,U]
    scores[b,t,s] = sum_u scale[u] * tanh(q[b,t,u] + k[b,s,u])
    attn = softmax(scores, axis=-1)     [B,Tq,Tk]
    context = attn @ value              [B,Tq,Dv]
    returns (context, attn)

Sharding: (b, tq-half) -> 8 cores, 128 query rows each; Tk local.

Algorithm (per core): tanh(x) ~= alpha*x + sum_{m odd<=9} c_m sin(m w x),
each sine separable -> 20 PE matmuls over u + exact linear term computed
as (alpha W2 scale) . v^T (4 matmuls, no k copy needed).  Harmonic factor
tensors built on DVE via Chebyshev step-2 recurrences over CONCATENATED
[sin|cos] buffers (halves op count; K ops are [128,2048], Q ops [128,512])
with base sin/cos on ACT (args < pi).  m=3 via (C4+-1) * base (D-trick).
Q side is pre-scaled by scale_u (S4 const) so taps are pure-imm scalings;
taps m=1,3 on DVE, m=5,7,9 on ACT (engine balance).

Schedule: PE warm-up matmul burst during the initial DMA wait flips the
HAM clock gate (1.2 -> 2.4 GHz) before real matmuls; keep-warm dummies
between score rounds.  Input DMAs are spread over 4 issue queues with
criticals (w2, vT) first.  Outputs in fp16 (halved DMA).
"""

from contextlib import ExitStack

import numpy as np

import concourse.bass as bass
import concourse.mybir as mybir
from concourse.bass_utils import run_bass_kernel_spmd

F32 = mybir.dt.float32
BF16 = mybir.dt.bfloat16
FP16 = mybir.dt.float16
I32 = mybir.dt.int32
AF = mybir.ActivationFunctionType
ALU = mybir.AluOpType

N_CORES = 8
B, TQ, TK, DQ, DV, U = 4, 256, 512, 512, 512, 256
T_ROWS = 128
UC = U // 128          # 2
DC = DQ // 128         # 4
SC = TK // 128         # 4

# tanh(x) ~= ALPHA*x + sum c_m sin(m pi x / L), m odd, |x|<=9.4
L_PER = 10.1
OMEGA = float(np.pi / L_PER)
ALPHA = 0.01535833903650663
MS = [1, 3, 5, 7, 9]
COEFS = [1.159928933795801, 0.30947520157694797, 0.11021886920747345,
         0.036057634532860455, 0.022497063758245502]
HALF_PI = float(np.pi / 2)

KW = UC * TK           # 1024 (K-side half-width)
QW = UC * 128          # 256  (Q-side half-width)
N_WARM = 30


def build_bass(debug: bool = False, race: bool = True) -> bass.Bass:
    nc = bass.Bass(detect_race_conditions=race)
    w2a_ext = nc.declare_dram_parameter("w2a", [128, 2 * U], FP16, isOutput=False)
    w2b_ext = nc.declare_dram_parameter("w2bx", [128, 2 * U], FP16, isOutput=False)
    vt_ext = [
        nc.declare_dram_parameter(f"vt{dc}", [128, TK], FP16, isOutput=False)
        for dc in range(DC)
    ]
    qt_ext = nc.declare_dram_parameter("qth", [128, DC * 128], FP16, isOutput=False)
    w1_ext = nc.declare_dram_parameter("w1h", [128, DC * U], FP16, isOutput=False)
    vb_ext = nc.declare_dram_parameter("vbb", [128, SC * DV], BF16, isOutput=False)
    U8 = mybir.dt.uint8
    CONST_W = 4 + 8 + 2 * 128 + 2 * DC   # hpi | scl | idb | w2s
    cst_ext = nc.declare_dram_parameter("constb", [128, CONST_W], U8, isOutput=False)
    ctx_ext = nc.declare_dram_parameter("context", [T_ROWS, DV], FP16, isOutput=True)
    attn_ext = nc.declare_dram_parameter("attn", [T_ROWS, TK], FP16, isOutput=True)
    dbg_ext = {}
    if debug:
        for nm, w, dt in [
            ("dbg_k1", 2 * KW, FP16), ("dbg_k3", 2 * KW, FP16),
            ("dbg_k9", 2 * KW, FP16), ("dbg_q1p", 2 * QW, FP16),
            ("dbg_q9", 2 * QW, FP16), ("dbg_a5", 2 * QW, FP16),
            ("dbg_e", TK, BF16), ("dbg_sums", 1, F32),
            ("dbg_etb", TK, BF16),
        ]:
            dbg_ext[nm] = nc.declare_dram_parameter(nm, [128, w], dt, isOutput=True)

    es = ExitStack()
    with es:
        _n = [0]

        def sb(shape, dt):
            _n[0] += 1
            return es.enter_context(nc.sbuf_tensor(f"sb{_n[0]}", shape, dt))

        # ---- SBUF ----
        w2b = sb([128, DC * U], FP16)
        vtb = sb([128, DC * TK], FP16)
        qtb = sb([128, DC * 128], FP16)
        w1b = sb([128, DC * U], FP16)
        vbf = sb([128, SC * DV], BF16)
        cstb = sb([128, CONST_W], U8)
        hpi = cstb[:, 0:4].bitcast(F32)                          # [128, 1]
        scl = cstb[:, 4:12].bitcast(F32)                         # [128, 2]
        idb = cstb[:, 12:268].bitcast(BF16)                      # [128, 128]
        w2ss = cstb[:, 268:276].bitcast(FP16)                    # [128, 4]
        wz = sb([128, 128], FP16)          # warm-up zeros
        # K-side concat factor tensors [u_p, sin(uc0|uc1) | cos(uc0|uc1)]
        K = {m: sb([128, 2 * KW], FP16) for m in MS}
        tmpK = sb([128, KW], FP16)
        C4K = sb([128, 2 * KW], FP16)      # [C4 | C4]
        DKc = sb([128, 2 * KW], FP16)      # [C4+1 | C4-1]
        tK = sb([128, 2 * KW], FP16)       # round scratch
        # Q side [u_p, sin | cos], prescaled by scale_u after m=1
        Q = {m: sb([128, 2 * QW], FP16) for m in MS}   # Q[1] raw; Q1P scaled
        Q1P = sb([128, 2 * QW], FP16)
        tmpQ = sb([128, QW], FP16)
        C4Q = sb([128, 2 * QW], FP16)
        DQc = sb([128, 2 * QW], FP16)
        tQ = sb([128, 2 * QW], FP16)
        A = {m: sb([128, 2 * QW], FP16) for m in MS}   # taps c_m * Q[m]
        E_bf = sb([128, TK], BF16)
        sums = sb([128, 1], F32)
        sums0 = sb([128, 1], F32)
        sums1 = sb([128, 1], F32)
        scr1 = sb([128, 1], F32)
        r_sb = sb([128, 1], F32)
        ETb = sb([128, TK], BF16)
        attn_h = sb([128, TK], FP16)
        ctx_h = sb([128, DV], FP16)

        # ---- PSUM (8 banks x 2KB) ----
        ringA = es.enter_context(nc.psum_tensor("ringA", [128, 2048], F32))
        ringB = es.enter_context(nc.psum_tensor("ringB", [128, 1536], F32))
        kps = ringA[:, 0:1024]                 # k proj [u, (uc,s)]
        scores_ps = ringA[:, 1024:1536]
        qps = ringA[:, 1536:1792]              # q proj [u, (uc,t)]
        warm_ps = ringB[:, 1024:1152]
        etps_bf = ringB[:, 0:256].bitcast(BF16)   # [128, 512] bf16 view
        etps_i32 = ringB[:, 0:256].bitcast(I32)   # [128, 256] i32 view
        ctxps = ringB[:, 512:1024]
        ETb_i32 = ETb.bitcast(I32)                # [128, 256]

        sem = lambda name: es.enter_context(nc.semaphore(name))
        s_w2a = sem("s_w2a")
        s_w2b = sem("s_w2b")
        s_vt = [sem(f"s_vt{dc}") for dc in range(DC)]
        s_qt = sem("s_qt")
        s_w1 = sem("s_w1")
        s_vbf = sem("s_vbf")
        s_cst = sem("s_cst")
        s_warm = sem("s_warm")
        s_kp = sem("s_kp")       # 2
        s_qp = sem("s_qp")       # 2
        s_act = sem("s_act")     # ACT bases: XsK1, XcK1, tq_s, tq_c
        s_tapD = sem("s_tapD")   # DVE taps A1, A3
        s_qr = sem("s_qr")       # DVE round completions (Q5, Q7, Q9)
        s_tapA = sem("s_tapA")   # ACT taps A5, A7, A9
        s_scores = sem("s_scores")
        s_exp = sem("s_exp")
        s_transp = sem("s_transp")  # 4
        s_etb = sem("s_etb")        # 2
        s_recip = sem("s_recip")
        s_ctxmm = sem("s_ctxmm")
        s_att = sem("s_att")
        s_ctxo = sem("s_ctxo")
        s_dout = sem("s_dout")

        c_of = {m: float(c) for m, c in zip(MS, COEFS)}

        with nc.Block() as block:

            @block.sync
            def _(sync):
                sync.dma_start(out=vtb[:, 0:512], in_=vt_ext[0][:, :]).then_inc(
                    s_vt[0], 16
                )
                sync.dma_start(
                    out=vtb[:, 1024:1536], in_=vt_ext[2][:, :]
                ).then_inc(s_vt[2], 16)
                sync.dma_start(out=w1b[:, :], in_=w1_ext[:, :]).then_inc(s_w1, 16)
                sync.wait_ge(s_att, 1)
                sync.dma_start(out=attn_ext[:, :], in_=attn_h[:, :]).then_inc(
                    s_dout, 16
                )

                if debug:
                    sync.wait_ge(s_ctxo, 1)
                    dbg_srcs = {
                        "dbg_k1": K[1], "dbg_k3": K[3], "dbg_k9": K[9],
                        "dbg_q1p": Q1P, "dbg_q9": Q[9], "dbg_a5": A[5],
                        "dbg_e": E_bf, "dbg_sums": sums, "dbg_etb": ETb,
                    }
                    for nm, src in dbg_srcs.items():
                        sync.dma_start(
                            out=dbg_ext[nm][:, :], in_=src[:, :]
                        ).then_inc(s_dout, 16)
                    sync.wait_ge(s_dout, 32 + 16 * len(dbg_srcs))
                else:
                    sync.wait_ge(s_dout, 32)

            @block.scalar
            def _(scalar):
                scalar.dma_start(out=w2b[:, 0 : 2 * U], in_=w2a_ext[:, :]).then_inc(
                    s_w2a, 16
                )
                scalar.dma_start(out=vtb[:, 512:1024], in_=vt_ext[1][:, :]).then_inc(
                    s_vt[1], 16
                )
                scalar.dma_start(
                    out=vtb[:, 1536:2048], in_=vt_ext[3][:, :]
                ).then_inc(s_vt[3], 16)
                scalar.dma_start(out=cstb[:, :], in_=cst_ext[:, :]).then_inc(
                    s_cst, 16
                )
                # preload the Sin table during the DMA wait
                scalar.wait_ge(s_warm, 1)
                scalar.activation(out=scr1[:, 0:1], in_=wz[:, 0:1], func=AF.Sin)
                # K bases into concat halves (args < pi; pi/2-bias cos valid
                # since |w*x| <= pi/2 with margin)
                scalar.wait_ge(s_kp, 2)
                scalar.activation(
                    out=K[1][:, 0:KW], in_=kps, func=AF.Sin, scale=OMEGA
                ).then_inc(s_act, 1)  # 1
                scalar.wait_ge(s_cst, 16)
                scalar.activation(
                    out=K[1][:, KW : 2 * KW], in_=kps, func=AF.Sin, scale=OMEGA,
                    bias=hpi[:, 0:1],
                ).then_inc(s_act, 1)  # 2
                scalar.wait_ge(s_qp, 2)
                scalar.activation(
                    out=Q[1][:, 0:QW], in_=qps, func=AF.Sin, scale=OMEGA
                ).then_inc(s_act, 1)  # 3
                scalar.activation(
                    out=Q[1][:, QW : 2 * QW], in_=qps, func=AF.Sin, scale=OMEGA,
                    bias=hpi[:, 0:1],
                ).then_inc(s_act, 1)  # 4
                # preload the Exp table now (off the critical tail)
                scalar.activation(out=scr1[:, 0:1], in_=wz[:, 0:1], func=AF.Exp)
                # ACT taps for m=5,7,9 as DVE rounds complete
                for i, m in enumerate(MS[2:]):
                    scalar.wait_ge(s_qr, i + 1)
                    scalar.activation(
                        out=A[m][:, :], in_=Q[m][:, :], func=AF.Copy,
                        scale=c_of[m],
                    ).then_inc(s_tapA, 1)
                # softmax exp + row sums
                scalar.wait_ge(s_scores, 1)
                scalar.activation(
                    out=E_bf[:, :], in_=scores_ps, func=AF.Exp,
                    accum_out=sums[:, 0:1],
                ).then_inc(s_exp, 1)
                scalar.activation(
                    out=E_bf[:, 0:1], in_=E_bf[:, 0:1], func=AF.Copy,
                ).then_inc(s_exp, 1)
                # attn normalize (fp16 out)
                scalar.wait_ge(s_recip, 1)
                scalar.activation(
                    out=attn_h[:, :], in_=E_bf[:, :], func=AF.Copy,
                    scale=r_sb[:, 0:1],
                ).then_inc(s_att, 1)
                scalar.wait_ge(s_ctxo, 1)
                scalar.dma_start(out=ctx_ext[:, :], in_=ctx_h[:, :]).then_inc(
                    s_dout, 16
                )

            @block.gpsimd
            def _(gpsimd):
                gpsimd.dma_start(
                    out=w2b[:, 2 * U : 4 * U], in_=w2b_ext[:, :]
                ).then_inc(s_w2b, 16)
                gpsimd.dma_start(out=qtb[:, :], in_=qt_ext[:, :]).then_inc(s_qt, 16)
                gpsimd.wait_ge(s_qp, 2)
                gpsimd.dma_start(out=vbf[:, :], in_=vb_ext[:, :]).then_inc(
                    s_vbf, 16
                )

            @block.vector
            def _(vector):
                vector.memset(wz[:, :], 0.0)
                vector.drain()
                vector.sem_inc(s_warm, 1)
                # K-side prep
                vector.wait_ge(s_act, 1)
                vector.tensor_tensor(
                    out=tmpK[:, :], in0=K[1][:, 0:KW], in1=K[1][:, 0:KW],
                    op=ALU.mult,
                )
                vector.tensor_scalar(
                    out=C4K[:, 0:KW], in0=tmpK[:, :], scalar1=-4.0, scalar2=2.0,
                    op0=ALU.mult, op1=ALU.add,
                )
                vector.tensor_scalar(
                    out=C4K[:, KW : 2 * KW], in0=tmpK[:, :], scalar1=-4.0,
                    scalar2=2.0, op0=ALU.mult, op1=ALU.add,
                )
                vector.tensor_scalar_add(
                    out=DKc[:, 0:KW], in0=C4K[:, 0:KW], scalar1=1.0
                )
                vector.tensor_scalar_add(
                    out=DKc[:, KW : 2 * KW], in0=C4K[:, 0:KW], scalar1=-1.0
                )
                # m=3 K: [s3|c3] = [C4+1|C4-1] * [s1|c1]
                vector.wait_ge(s_act, 2)
                vector.tensor_tensor(
                    out=K[3][:, :], in0=DKc[:, :], in1=K[1][:, :], op=ALU.mult
                )
                # Q-side prep
                vector.wait_ge(s_act, 3)
                vector.tensor_tensor(
                    out=tmpQ[:, :], in0=Q[1][:, 0:QW], in1=Q[1][:, 0:QW],
                    op=ALU.mult,
                )
                vector.tensor_scalar(
                    out=C4Q[:, 0:QW], in0=tmpQ[:, :], scalar1=-4.0, scalar2=2.0,
                    op0=ALU.mult, op1=ALU.add,
                )
                vector.tensor_scalar(
                    out=C4Q[:, QW : 2 * QW], in0=tmpQ[:, :], scalar1=-4.0,
                    scalar2=2.0, op0=ALU.mult, op1=ALU.add,
                )
                vector.tensor_scalar_add(
                    out=DQc[:, 0:QW], in0=C4Q[:, 0:QW], scalar1=1.0
                )
                vector.tensor_scalar_add(
                    out=DQc[:, QW : 2 * QW], in0=C4Q[:, 0:QW], scalar1=-1.0
                )
                # prescale by scale_u (S4 const), tap m=1, m=3 chain + tap
                vector.wait_ge(s_act, 4)
                vector.wait_ge(s_cst, 16)
                for blk in range(4):
                    vector.tensor_scalar_mul(
                        out=Q1P[:, blk * 128 : (blk + 1) * 128],
                        in0=Q[1][:, blk * 128 : (blk + 1) * 128],
                        scalar1=scl[:, blk % 2 : blk % 2 + 1],
                    )
                vector.tensor_scalar_mul(
                    out=A[1][:, :], in0=Q1P[:, :], scalar1=c_of[1]
                ).then_inc(s_tapD, 1)
                vector.tensor_tensor(
                    out=Q[3][:, :], in0=DQc[:, :], in1=Q1P[:, :], op=ALU.mult
                )
                vector.tensor_scalar_mul(
                    out=A[3][:, :], in0=Q[3][:, :], scalar1=c_of[3]
                ).then_inc(s_tapD, 1)
                # rounds m = 5, 7, 9 on concat buffers
                for j, m in enumerate(MS[2:]):
                    p2, p4 = m - 2, m - 4
                    q_p4 = Q1P if p4 == 1 else Q[p4]
                    vector.tensor_tensor(
                        out=tK[:, :], in0=C4K[:, :], in1=K[p2][:, :], op=ALU.mult
                    )
                    vector.tensor_tensor(
                        out=tQ[:, :], in0=C4Q[:, :], in1=Q[p2][:, :], op=ALU.mult
                    )
                    vector.tensor_tensor(
                        out=K[m][:, :], in0=tK[:, :], in1=K[p4][:, :],
                        op=ALU.subtract,
                    )
                    vector.tensor_tensor(
                        out=Q[m][:, :], in0=tQ[:, :], in1=q_p4[:, :],
                        op=ALU.subtract,
                    ).then_inc(s_qr, 1)
                # 1/sums (ahead of evacs: unblocks the attn path)
                vector.wait_ge(s_exp, 2)
                vector.reciprocal(out=r_sb[:, :], in_=sums[:, :])
                vector.drain()
                vector.sem_inc(s_recip, 1)
                # ET evac (bf16 as int32: half the elements)
                vector.wait_ge(s_transp, 2)
                vector.tensor_copy(out=ETb_i32[:, 0:128], in_=etps_i32[:, 0:128])
                vector.drain()
                vector.sem_inc(s_etb, 1)
                vector.wait_ge(s_transp, 4)
                vector.tensor_copy(out=ETb_i32[:, 128:256], in_=etps_i32[:, 128:256])
                vector.drain()
                vector.sem_inc(s_etb, 1)
                # ctx normalize (fp16 out)
                vector.wait_ge(s_ctxmm, 1)
                vector.tensor_scalar_mul(
                    out=ctx_h[:, :], in0=ctxps, scalar1=r_sb[:, 0:1]
                ).then_inc(s_ctxo, 1)

            @block.tensor
            def _(tensor):
                # HAM warm-up burst on zeros during the input DMA wait
                tensor.wait_ge(s_warm, 1)
                for _ in range(N_WARM):
                    tensor.matmul(
                        out=warm_ps, lhsT=wz[:, :], rhs=wz[:, :],
                        start=True, stop=True,
                    )
                # k projection (interleaved uc groups, per-dc-pair waits)
                for dc in range(DC):
                    if dc == 0:
                        tensor.wait_ge(s_w2a, 16)
                    elif dc == 2:
                        tensor.wait_ge(s_w2b, 16)
                    tensor.wait_ge(s_vt[dc], 16)
                    for uc in range(UC):
                        ins = tensor.matmul(
                            out=kps[:, uc * TK : (uc + 1) * TK],
                            lhsT=w2b[:, dc * U + uc * 128 : dc * U + uc * 128 + 128],
                            rhs=vtb[:, dc * TK : (dc + 1) * TK],
                            start=(dc == 0),
                            stop=(dc == DC - 1),
                        )
                        if dc == DC - 1:
                            ins.then_inc(s_kp, 1)
                # q projection (uc groups sequential: one open group per bank)
                tensor.wait_ge(s_qt, 16)
                tensor.wait_ge(s_w1, 16)
                for uc in range(UC):
                    for dc in range(DC):
                        ins = tensor.matmul(
                            out=qps[:, uc * 128 : (uc + 1) * 128],
                            lhsT=w1b[:, dc * U + uc * 128 : dc * U + uc * 128 + 128],
                            rhs=qtb[:, dc * 128 : (dc + 1) * 128],
                            start=(dc == 0),
                            stop=(dc == DC - 1),
                        )
                    ins.then_inc(s_qp, 1)
                # scores: exact linear term first (w2s . vT, const along t)
                tensor.wait_ge(s_cst, 16)
                for dc in range(DC):
                    tensor.matmul(
                        out=scores_ps,
                        lhsT=w2ss[:, dc : dc + 1].broadcast_to([128, 128]),
                        rhs=vtb[:, dc * TK : (dc + 1) * TK],
                        start=(dc == 0),
                        stop=False,
                    )
                # harmonics: lhsT = A[m] tiles, rhs = K[m] slices
                for i, m in enumerate(MS):
                    # keep-warm dummies before the tap wait
                    for _ in range(2):
                        tensor.matmul(
                            out=warm_ps, lhsT=wz[:, :], rhs=wz[:, :],
                            start=True, stop=True,
                        )
                    if m == 1:
                        tensor.wait_ge(s_act, 2)
                        tensor.wait_ge(s_tapD, 1)
                    elif m == 3:
                        tensor.wait_ge(s_tapD, 2)
                    else:
                        tensor.wait_ge(s_tapA, i - 1)
                    # A layout [As_uc0|As_uc1|Ac_uc0|Ac_uc1]
                    # K layout [Ks_uc0|Ks_uc1|Kc_uc0|Kc_uc1] (512 each)
                    for kind in range(2):   # 0: As x Kc, 1: Ac x Ks
                        for uc in range(UC):
                            lhs = A[m][:, (kind * 2 + uc) * 128 :
                                       (kind * 2 + uc) * 128 + 128]
                            rhs_off = (1 - kind) * KW + uc * TK
                            last = (m == MS[-1]) and (kind == 1) and (uc == UC - 1)
                            ins = tensor.matmul(
                                out=scores_ps,
                                lhsT=lhs,
                                rhs=K[m][:, rhs_off : rhs_off + TK],
                                start=False,
                                stop=last,
                            )
                            if last:
                                ins.then_inc(s_scores, 1)
                # transposes of E for ctx (per exp chunk)
                tensor.matmul(
                    out=warm_ps, lhsT=wz[:, :], rhs=wz[:, :], start=True, stop=True
                )
                tensor.matmul(
                    out=warm_ps, lhsT=wz[:, :], rhs=wz[:, :], start=True, stop=True
                )
                for sc in range(SC):
                    if sc == 0:
                        tensor.wait_ge(s_exp, 1)
                    elif sc == 2:
                        tensor.wait_ge(s_exp, 2)
                    tensor.transpose(
                        out=etps_bf[:, sc * 128 : (sc + 1) * 128],
                        in_=E_bf[:, sc * 128 : (sc + 1) * 128],
                        identity=idb[:, :],
                    ).then_inc(s_transp, 1)
                # context (chunk-pipelined on ET evacs)
                for _ in range(2):
                    tensor.matmul(
                        out=warm_ps, lhsT=wz[:, :], rhs=wz[:, :],
                        start=True, stop=True,
                    )
                tensor.wait_ge(s_vbf, 16)
                for sc in range(SC):
                    if sc == 0:
                        tensor.wait_ge(s_etb, 1)
                    elif sc == 2:
                        tensor.wait_ge(s_etb, 2)
                    ins = tensor.matmul(
                        out=ctxps,
                        lhsT=ETb[:, sc * 128 : (sc + 1) * 128],
                        rhs=vbf[:, sc * DV : (sc + 1) * DV],
                        start=(sc == 0),
                        stop=(sc == SC - 1),
                    )
                    if sc == SC - 1:
                        ins.then_inc(s_ctxmm, 1)

    return nc


_NC = None


def _get_nc() -> bass.Bass:
    global _NC
    if _NC is None:
        _NC = build_bass()
    return _NC


def make_in_maps(query, value, W1, W2, scale):
    import ml_dtypes

    bf = ml_dtypes.bfloat16
    fh = np.float16
    query = np.asarray(query, dtype=np.float32)
    value = np.asarray(value, dtype=np.float32)
    W1 = np.asarray(W1, np.float32)
    W2 = np.asarray(W2, np.float32)
    scale = np.asarray(scale, np.float32)
    # pack [D, X] operands into SBUF layout [128, (chunk, x)]
    pk = lambda a: np.ascontiguousarray(
        a.reshape(4, 128, a.shape[1]).transpose(1, 0, 2).reshape(128, -1)
    )
    w1h = pk(W1.astype(fh))
    w2h = pk(W2.astype(fh))
    scl = np.ascontiguousarray(scale.reshape(UC, 128).T.astype(np.float32))
    w2s = np.ascontiguousarray(
        (ALPHA * (W2 @ scale)).astype(fh).reshape(DC, 128).T
    )
    constb = np.ascontiguousarray(
        np.concatenate(
            [
                np.full((128, 1), np.pi / 2, np.float32).view(np.uint8),
                scl.view(np.uint8),
                np.eye(128).astype(bf).view(np.uint8),
                w2s.view(np.uint8),
            ],
            axis=1,
        )
    )
    in_maps = []
    for c in range(N_CORES):
        b, th = c // 2, c % 2
        qloc = query[b, th * T_ROWS : (th + 1) * T_ROWS, :]
        vloc = value[b]
        vth = pk(vloc.T.astype(fh))
        im = {
            "w2a": np.ascontiguousarray(w2h[:, 0:512]),
            "w2bx": np.ascontiguousarray(w2h[:, 512:1024]),
            "qth": pk(qloc.T.astype(fh)),
            "w1h": w1h,
            "vbb": pk(vloc.astype(bf)),
            "constb": constb,
        }
        for dc in range(DC):
            im[f"vt{dc}"] = np.ascontiguousarray(vth[:, dc * 512 : (dc + 1) * 512])
        in_maps.append(im)
    return in_maps


def assemble(results):
    context = np.empty((B, TQ, DV), dtype=np.float32)
    attn = np.empty((B, TQ, TK), dtype=np.float32)
    for c in range(N_CORES):
        b, th = c // 2, c % 2
        context[b, th * T_ROWS : (th + 1) * T_ROWS, :] = results[c][
            "context"
        ].astype(np.float32)
        attn[b, th * T_ROWS : (th + 1) * T_ROWS, :] = results[c]["attn"].astype(
            np.float32
        )
    return context, attn


def kernel(query, value, W1, W2, scale):
    nc = _get_nc()
    in_maps = make_in_maps(query, value, W1, W2, scale)
    res = run_bass_kernel_spmd(nc, in_maps, core_ids=list(range(N_CORES)))
    return assemble(res.results)
